# revision 1
# baseline (speedup 1.0000x reference)
"""GCL_skip_global distributed Trainium2 kernel.

Strategy (per sharding hint): 1D node partition across 8 cores (6250 nodes
each, padded to 6272). Each core projects its node slice (h@wh, s@ws,
m@wm+bias) with float32r matmuls, AllGathers the projected features, then
aggregates edges owned by dst via indirect row gathers + selection-matrix
matmuls (segment_sum as PE work), with both norm scalings folded into the
selection matrix values host-side. Final: relu(agg_g + agg_f + mproj).
"""
import sys
sys.path.insert(0, '/opt/trn_rl_repo')
import numpy as np
from concourse import bass, mybir, bacc
import concourse.tile as tile

F32 = mybir.dt.float32
F32R = mybir.dt.float32r
BF16 = mybir.dt.bfloat16
I32 = mybir.dt.int32
I16 = mybir.dt.int16
import ml_dtypes
NP_BF16 = ml_dtypes.bfloat16

CORES = 8
N = 50000
D = 512
NPC = N // CORES           # 6250 nodes per core
NPAD = 6272                # padded to 49*128
MT = NPAD // 128           # 49 m-tiles
KT = D // 128              # 4 k-tiles
LOWCAP = 640               # low-half edge slots per tile (blocks 0-4)
HIGHCAP = 384              # high-half edge slots per tile (blocks 5-7)
LBLK = LOWCAP // 128       # 5
BPT = (LOWCAP + HIGHCAP) // 128   # 8 blocks per tile
TRASH = NPC + 2            # scatter target for pad slots (< NPAD)


# ---------------------------------------------------------------- host prep

def _wrap16(vals):
    """[n] int16 -> [128, n/16]: index j at (j%16, j//16), replicated to 8 Q7 cores."""
    return np.tile(vals.reshape(-1, 16).T, (8, 1))


def _pack_one(dst, src, scale_d, scale_s, core):
    """Pack one (graph, core)'s edges into tiles of 8 blocks: 5 low + 3 high.

    "low" edges have source feature row < 32768 in the AllGathered buffer
    (int16-addressable directly); "high" edges are gathered from a +32768
    base.  Each tile covers <=128 consecutive dst nodes with <=LOWCAP low
    and <=HIGHCAP high edges; low edges occupy slot positions [0, LOWCAP),
    high [LOWCAP, LOWCAP+HIGHCAP).  Pad positions gather row 0 of their
    half with a zero one-hot column.

    Returns (idxlo [640], idxhi [384] per tile concatenated, oh, rid, nt).
    """
    lo = core * NPC
    sel = (dst >= lo) & (dst < lo + NPC)
    d = (dst[sel].astype(np.int64) - lo)
    s = src[sel].astype(np.int64)
    w = (scale_d[dst[sel]] * scale_s[s]).astype(np.float32)

    deg = np.bincount(d, minlength=NPC)
    zd = np.where(deg == 0)[0]
    if len(zd):
        d = np.concatenate([d, zd])
        s = np.concatenate([s, np.zeros(len(zd), np.int64)])
        w = np.concatenate([w, np.zeros(len(zd), np.float32)])

    gsrc = (s // NPC) * NPAD + (s % NPC)                 # row in P_full
    ishigh = (gsrc >= 32768).astype(np.int64)
    # sort edges by (dst, ishigh) so each node's low edges precede its high
    order = np.lexsort((ishigh, d))
    d, gsrc, w, ishigh = d[order], gsrc[order], w[order], ishigh[order]

    deglo = np.bincount(d[ishigh == 0], minlength=NPC)
    deghi = np.bincount(d[ishigh == 1], minlength=NPC)
    cumlo = np.zeros(NPC + 1, np.int64); np.cumsum(deglo, out=cumlo[1:])
    cumhi = np.zeros(NPC + 1, np.int64); np.cumsum(deghi, out=cumhi[1:])
    assert deglo.max() <= LOWCAP and deghi.max() <= HIGHCAP

    bounds = [0]
    n0 = 0
    while n0 < NPC:
        n1a = int(np.searchsorted(cumlo, cumlo[n0] + LOWCAP, side='right')) - 1
        n1b = int(np.searchsorted(cumhi, cumhi[n0] + HIGHCAP, side='right')) - 1
        n1 = min(max(min(n1a, n1b), n0 + 1), n0 + 128, NPC)
        bounds.append(n1)
        n0 = n1
    bounds = np.asarray(bounds, np.int64)
    nt = len(bounds) - 1

    nodes_per_tile = np.diff(bounds)
    tile_of_node = np.repeat(np.arange(nt), nodes_per_tile)
    t_e = tile_of_node[d]
    # position within tile's low / high run
    cumlo_b = cumlo[bounds[:-1]]
    cumhi_b = cumhi[bounds[:-1]]
    # rank of edge within its (tile, half): edges sorted by (d, ishigh);
    # running index over the half within the tile:
    elo = np.cumsum(ishigh == 0) - 1                     # rank among low edges overall
    ehi = np.cumsum(ishigh == 1) - 1
    jlo = elo - cumlo_b[t_e]                             # only valid where low
    jhi = ehi - cumhi_b[t_e]
    pos = np.where(ishigh == 0, jlo, LOWCAP + jhi)
    assert pos.min() >= 0 and (pos < LOWCAP + HIGHCAP).all()
    b = pos // 128
    p = pos % 128
    blk = t_e * BPT + b
    slot = d - bounds[t_e]
    assert slot.max() < 128

    idxlo = np.zeros((nt, LOWCAP), np.int16)
    idxhi = np.zeros((nt, HIGHCAP), np.int16)
    mlow = ishigh == 0
    idxlo[t_e[mlow], pos[mlow]] = gsrc[mlow].astype(np.int16)
    idxhi[t_e[~mlow], pos[~mlow] - LOWCAP] = (gsrc[~mlow] - 32768).astype(np.int16)
    assert (idxlo >= 0).all() and (idxhi >= 0).all()

    oh = np.zeros((128, nt * BPT * 128), NP_BF16)
    oh[p, blk * 128 + slot] = w.astype(NP_BF16)
    rid = bounds[:-1, None] + np.arange(128)[None, :]
    rid = np.where(rid < bounds[1:, None], rid, TRASH).astype(np.int32)
    return idxlo, idxhi, oh, rid, nt


def _pad_tiles(idxlo, idxhi, oh, rid, nt, nt_common):
    il = np.zeros((nt_common, LOWCAP), np.int16)
    il[:nt] = idxlo
    ih = np.zeros((nt_common, HIGHCAP), np.int16)
    ih[:nt] = idxhi
    # wrap each tile's run and concatenate along free dim
    il16 = np.concatenate([_wrap16(il[t]) for t in range(nt_common)], axis=1)
    ih16 = np.concatenate([_wrap16(ih[t]) for t in range(nt_common)], axis=1)
    oh2 = np.zeros((128, nt_common * BPT * 128), NP_BF16)
    oh2[:, :oh.shape[1]] = oh
    rid2 = np.full((128, nt_common), TRASH, np.int32)
    rid2[:, :nt] = rid.T
    return il16, ih16, oh2, rid2


def prep_inputs(inp):
    """Full inputs -> (per-core input maps, meta)."""
    h, s, m = (np.asarray(inp[k], np.float32) for k in ('h', 's', 'm'))
    norm_g = np.asarray(inp['norm_g'], np.float32).reshape(-1)
    norm_f = np.asarray(inp['norm_f'], np.float32).reshape(-1)
    src_g = np.asarray(inp['src_g']); dst_g = np.asarray(inp['dst_g'])
    src_f = np.asarray(inp['src_f']); dst_f = np.asarray(inp['dst_f'])
    wh, ws, wm = (np.asarray(inp[k], np.float32) for k in ('wh', 'ws', 'wm'))
    bias = (np.asarray(inp['bh']) + np.asarray(inp['bs']) + np.asarray(inp['bm'])).astype(np.float32)

    def wr(wmat, npdt):  # [D, D] -> [128, KT*D] : wr[p, k*D+j] = w[k*128+p, j]
        return np.ascontiguousarray(
            wmat.reshape(KT, 128, D).transpose(1, 0, 2).reshape(128, KT * D).astype(npdt))

    whr, wsr, wmr = wr(wh, NP_BF16), wr(ws, NP_BF16), wr(wm, np.float32)
    biasrep = np.broadcast_to(bias, (128, D)).copy()

    packs_g, packs_f = [], []
    for k in range(CORES):
        packs_g.append(_pack_one(dst_g, src_g, norm_g, norm_g, k))
        packs_f.append(_pack_one(dst_f, src_f, norm_f, norm_f, k))
    ntg = max(p[4] for p in packs_g)
    ntf = max(p[4] for p in packs_f)

    in_maps = []
    for k in range(CORES):
        sl = slice(k * NPC, (k + 1) * NPC)

        def tpad(x, npdt):  # [NPC, D] -> [MT, 128, KT*128]: SBUF image per m-tile
            xp = np.zeros((NPAD, D), np.float32)
            xp[:NPC] = x[sl]
            # hTi[mt, p, kt*128+j] = xp[mt*128+j, kt*128+p]
            return np.ascontiguousarray(
                xp.reshape(MT, 128, KT, 128).transpose(0, 3, 2, 1).reshape(
                    MT, 128, KT * 128).astype(npdt))

        ilg, ihg, ohg, ridg = _pad_tiles(*packs_g[k][:4], packs_g[k][4], ntg)
        ilf, ihf, ohf, ridf = _pad_tiles(*packs_f[k][:4], packs_f[k][4], ntf)
        in_maps.append({
            'hT': tpad(h, NP_BF16), 'sT': tpad(s, NP_BF16), 'mT': tpad(m, np.float32),
            'whr': whr, 'wsr': wsr, 'wmr': wmr, 'biasrep': biasrep,
            'ilg': ilg, 'ihg': ihg, 'ohg': ohg, 'ridg': ridg,
            'ilf': ilf, 'ihf': ihf, 'ohf': ohf, 'ridf': ridf,
        })
    return in_maps, (ntg, ntf)


# ---------------------------------------------------------------- device code

def build_nc(ntg, ntf, stages=("s1", "ag", "s3g", "s3f", "fin"), reps=1, scatter_plain=False, gather_plain=False, gat_bufs=6, ps3_bufs=4, fin_bufs=3):
    stages = frozenset(stages)
    nc = bacc.Bacc("TRN2", target_bir_lowering=False, debug=False)

    hT = nc.dram_tensor("hT", [MT, 128, KT * 128], BF16, kind="ExternalInput")
    sT = nc.dram_tensor("sT", [MT, 128, KT * 128], BF16, kind="ExternalInput")
    mT = nc.dram_tensor("mT", [MT, 128, KT * 128], F32R, kind="ExternalInput")
    whr = nc.dram_tensor("whr", [128, KT * D], BF16, kind="ExternalInput")
    wsr = nc.dram_tensor("wsr", [128, KT * D], BF16, kind="ExternalInput")
    wmr = nc.dram_tensor("wmr", [128, KT * D], F32R, kind="ExternalInput")
    biasrep = nc.dram_tensor("biasrep", [128, D], F32, kind="ExternalInput")
    ilg = nc.dram_tensor("ilg", [128, ntg * LOWCAP // 16], I16, kind="ExternalInput")
    ihg = nc.dram_tensor("ihg", [128, ntg * HIGHCAP // 16], I16, kind="ExternalInput")
    ohg = nc.dram_tensor("ohg", [128, ntg * BPT * 128], BF16, kind="ExternalInput")
    ridg = nc.dram_tensor("ridg", [128, ntg], I32, kind="ExternalInput")
    ilf = nc.dram_tensor("ilf", [128, ntf * LOWCAP // 16], I16, kind="ExternalInput")
    ihf = nc.dram_tensor("ihf", [128, ntf * HIGHCAP // 16], I16, kind="ExternalInput")
    ohf = nc.dram_tensor("ohf", [128, ntf * BPT * 128], BF16, kind="ExternalInput")
    ridf = nc.dram_tensor("ridf", [128, ntf], I32, kind="ExternalInput")
    out = nc.dram_tensor("out", [NPAD, D], F32, kind="ExternalOutput")

    pg_b = nc.dram_tensor("pg_b", [NPAD, D], BF16)
    pf_b = nc.dram_tensor("pf_b", [NPAD, D], BF16)
    pg_full = nc.dram_tensor("pg_full", [CORES * NPAD, D], BF16, addr_space="Shared")
    pf_full = nc.dram_tensor("pf_full", [CORES * NPAD, D], BF16, addr_space="Shared")
    mproj = nc.dram_tensor("mproj", [NPAD, D], F32)
    aggg = nc.dram_tensor("aggg", [NPAD, D], F32)
    aggf = nc.dram_tensor("aggf", [NPAD, D], F32)

    with tile.TileContext(nc) as tc:
        with (
            tc.tile_pool(name="w", bufs=2) as wp,
            tc.tile_pool(name="lhs", bufs=3) as lp,
            tc.tile_pool(name="gat", bufs=gat_bufs) as gp,
            tc.tile_pool(name="ohp", bufs=6) as op_,
            tc.tile_pool(name="epi", bufs=4) as ep,
            tc.tile_pool(name="sml", bufs=4) as sp,
            tc.tile_pool(name="fin", bufs=fin_bufs) as fp,
            tc.tile_pool(name="ps1", bufs=4, space="PSUM") as ps1,
            tc.tile_pool(name="ps3", bufs=ps3_bufs, space="PSUM") as ps3,
        ):
            bias_sb = wp.tile([128, D], F32, tag="bias", bufs=1)
            nc.sync.dma_start(out=bias_sb[:], in_=biasrep[:, :])

            for _rep in range(reps):

                # ---- stage 1: projections -> bounce buffers
                def project(xT, wrT, dst_dram, add_bias):
                    mmdt = F32R if add_bias else BF16
                    w_sb = wp.tile([128, KT * D], mmdt, tag="w")
                    nc.sync.dma_start(out=w_sb[:], in_=wrT[:, :])
                    for mt in range(MT):
                        lhs = lp.tile([128, KT * 128], mmdt, tag="lhs")
                        nc.sync.dma_start(out=lhs[:, :], in_=xT[mt, :, :])
                        psum = ps1.tile([128, D], F32)
                        for kt in range(KT):
                            nc.tensor.matmul(
                                out=psum[:], lhsT=lhs[:, kt * 128:(kt + 1) * 128],
                                rhs=w_sb[:, kt * D:(kt + 1) * D],
                                start=(kt == 0), stop=(kt == KT - 1))
                        o = ep.tile([128, D], BF16 if not add_bias else F32, tag="epi1")
                        if add_bias:
                            nc.vector.tensor_add(o[:], psum[:], bias_sb[:])
                        else:
                            nc.vector.tensor_copy(o[:], psum[:])
                        nc.sync.dma_start(
                            out=dst_dram[mt * 128:(mt + 1) * 128, :], in_=o[:])

                if "s1" in stages:
                    project(hT, whr, pg_b, False)

                if "ag" in stages:
                    # ---- AllGather graph g (overlaps with s/m projections)
                    nc.gpsimd.collective_compute(
                        "AllGather", mybir.AluOpType.bypass,
                        replica_groups=[list(range(CORES))],
                        ins=[pg_b.ap().opt()], outs=[pg_full.ap().opt()])

                if "s1" in stages:
                    project(sT, wsr, pf_b, False)

                if "ag" in stages:
                    nc.gpsimd.collective_compute(
                        "AllGather", mybir.AluOpType.bypass,
                        replica_groups=[list(range(CORES))],
                        ins=[pf_b.ap().opt()], outs=[pf_full.ap().opt()])

                if "s1" in stages:
                    project(mT, wmr, mproj, True)

                # ---- stage 3: aggregation per graph
                def aggregate(nt, il_dram, ih_dram, oh_dram, rid_dram, pfull_dram, agg_dram, tag):
                    LC16, HC16 = LOWCAP // 16, HIGHCAP // 16
                    il_sb = wp.tile([128, nt * LC16], I16, tag=f"il{tag}", bufs=1)
                    nc.sync.dma_start(out=il_sb[:], in_=il_dram[:, :])
                    ih_sb = wp.tile([128, nt * HC16], I16, tag=f"ih{tag}", bufs=1)
                    nc.sync.dma_start(out=ih_sb[:], in_=ih_dram[:, :])
                    rid_sb = wp.tile([128, nt], I32, tag=f"rid{tag}", bufs=1)
                    nc.sync.dma_start(out=rid_sb[:], in_=rid_dram[:, :])
                    for t in range(nt):
                        psum = ps3.tile([128, D], F32)
                        o = op_.tile([128, BPT * 128], BF16, tag="oh")
                        nc.sync.dma_start(
                            out=o[:], in_=oh_dram[:, t * BPT * 128:(t + 1) * BPT * 128])
                        g = gp.tile([128, BPT, D], BF16, tag="gather")
                        nc.gpsimd.dma_gather(
                            out_ap=g[:, 0:LBLK, :], in_ap=pfull_dram.ap()[:, :],
                            idxs_ap=il_sb[:, t * LC16:(t + 1) * LC16],
                            num_idxs=LOWCAP, num_idxs_reg=LOWCAP, elem_size=D)
                        hbase = 32768 if CORES * NPAD > 32768 else 0
                        nc.gpsimd.dma_gather(
                            out_ap=g[:, LBLK:BPT, :], in_ap=pfull_dram.ap()[hbase:, :],
                            idxs_ap=ih_sb[:, t * HC16:(t + 1) * HC16],
                            num_idxs=HIGHCAP, num_idxs_reg=HIGHCAP, elem_size=D)
                        for b in range(BPT):
                            nc.tensor.matmul(out=psum[:], lhsT=o[:, b * 128:(b + 1) * 128],
                                             rhs=g[:, b, :], start=(b == 0), stop=(b == BPT - 1))
                        eo = ep.tile([128, D], F32, tag="epi3")
                        nc.vector.tensor_copy(eo[:], psum[:])
                        nc.gpsimd.indirect_dma_start(
                            out=agg_dram[:, :],
                            out_offset=bass.IndirectOffsetOnAxis(ap=rid_sb[:, t:t + 1], axis=0),
                            in_=eo[:], in_offset=None)

                if "s3g" in stages:
                    aggregate(ntg, ilg, ihg, ohg, ridg, pg_full, aggg, "g")
                if "s3f" in stages:
                    aggregate(ntf, ilf, ihf, ohf, ridf, pf_full, aggf, "f")

                # ---- final: out = relu(aggg + aggf + mproj)
                for mt in range(MT) if "fin" in stages else []:
                    sl = slice(mt * 128, (mt + 1) * 128)
                    a = fp.tile([128, D], F32, tag="fa")
                    b_ = fp.tile([128, D], F32, tag="fb")
                    c = fp.tile([128, D], F32, tag="fc")
                    nc.sync.dma_start(out=a[:], in_=aggg[sl, :])
                    nc.sync.dma_start(out=b_[:], in_=aggf[sl, :])
                    nc.sync.dma_start(out=c[:], in_=mproj[sl, :])
                    nc.vector.tensor_add(a[:], a[:], b_[:])
                    nc.vector.tensor_add(a[:], a[:], c[:])
                    nc.vector.tensor_scalar_max(a[:], a[:], 0.0)
                    nc.sync.dma_start(out=out[sl, :], in_=a[:])

    nc.compile()
    return nc


def postprocess(results):
    return np.concatenate([results[k]["out"][:NPC] for k in range(CORES)], axis=0)


# ---------------------------------------------------------------- entry point

_NC_CACHE = {}


def _get_nc(ntg, ntf):
    key = (ntg, ntf)
    if key not in _NC_CACHE:
        _NC_CACHE[key] = build_nc(ntg, ntf)
    return _NC_CACHE[key]


def kernel(**inputs) -> np.ndarray:
    from concourse.bass_utils import run_bass_kernel_spmd
    in_maps, (ntg, ntf) = prep_inputs(inputs)
    nc = _get_nc(ntg, ntf)
    res = run_bass_kernel_spmd(nc, in_maps, core_ids=list(range(CORES)))
    return postprocess(res.results)


# ------------------------------------------------------- timing helper (test)

def _compile_jit(nc):
    """Mimic bass2jax.run_bass_via_pjrt but return a reusable jitted callable
    (no donation) so repeated dispatch can be timed."""
    import jax
    from jax.sharding import Mesh, PartitionSpec, NamedSharding
    from jax.experimental.shard_map import shard_map
    from concourse import bass2jax

    bass2jax.install_neuronx_cc_hook()
    in_names, out_names, out_avals, zero_outs = [], [], [], []
    for alloc in nc.m.functions[0].allocations:
        if not isinstance(alloc, mybir.MemoryLocationSet):
            continue
        name = alloc.memorylocations[0].name
        if alloc.kind == "ExternalInput":
            if name != "partition_id":
                in_names.append(name)
        elif alloc.kind == "ExternalOutput":
            out_names.append(name)
            shape = tuple(alloc.tensor_shape)
            dtype = mybir.dt.np(alloc.dtype)
            out_avals.append(jax.core.ShapedArray(shape, dtype))
            zero_outs.append(np.zeros(shape, dtype))
    n_params = len(in_names)
    all_names = in_names + out_names + ["partition_id"]

    def _body(*args):
        operands = list(args) + [bass2jax.partition_id_tensor()]
        outs = bass2jax._bass_exec_p.bind(
            *operands, out_avals=tuple(out_avals), in_names=tuple(all_names),
            out_names=tuple(out_names), lowering_input_output_aliases=(),
            sim_require_finite=True, sim_require_nnan=True, nc=nc)
        return tuple(outs)

    devices = jax.devices()[:CORES]
    mesh = Mesh(np.asarray(devices), ("core",))
    n_outs = len(out_names)
    in_specs = (PartitionSpec("core"),) * (n_params + n_outs)
    out_specs = (PartitionSpec("core"),) * n_outs
    fn = jax.jit(shard_map(_body, mesh=mesh, in_specs=in_specs,
                           out_specs=out_specs, check_rep=False), keep_unused=True)
    sharding = NamedSharding(mesh, PartitionSpec("core"))
    return fn, in_names, zero_outs, sharding


def _timed_min(fn, args, n=15):
    import jax, time
    o = fn(*args)
    jax.block_until_ready(o)
    ts = []
    for _ in range(n):
        t0 = time.perf_counter()
        o = fn(*args)
        jax.block_until_ready(o)
        ts.append(time.perf_counter() - t0)
    return float(np.min(ts))


def _build_baseline_nc():
    nc = bacc.Bacc("TRN2", target_bir_lowering=False, debug=False)
    x = nc.dram_tensor("x", [128, 128], F32, kind="ExternalInput")
    y = nc.dram_tensor("y", [128, 128], F32, kind="ExternalOutput")
    with tile.TileContext(nc) as tc:
        with tc.tile_pool(name="sb", bufs=1) as sb:
            t = sb.tile([128, 128], F32)
            nc.sync.dma_start(out=t[:], in_=x[:, :])
            nc.vector.tensor_scalar_mul(t[:], t[:], 2.0)
            nc.sync.dma_start(out=y[:, :], in_=t[:])
    nc.compile()
    return nc


def measure_hw_ns(inputs, n=15):
    """Estimate on-device kernel time: min-wall(kernel) - min-wall(baseline)."""
    import jax
    in_maps, (ntg, ntf) = prep_inputs(inputs)
    nc = _get_nc(ntg, ntf)
    fn, in_names, zero_outs, sh = _compile_jit(nc)
    args = [jax.device_put(
        np.concatenate([m[nm] for m in in_maps], axis=0), sh) for nm in in_names]
    args += [jax.device_put(
        np.zeros((CORES * z.shape[0], *z.shape[1:]), z.dtype), sh) for z in zero_outs]

    fnb, bin_names, bzero, shb = _compile_jit(_build_baseline_nc())
    argsb = [jax.device_put(np.zeros((CORES * 128, 128), np.float32), shb)]
    argsb += [jax.device_put(np.zeros((CORES * z.shape[0], *z.shape[1:]), z.dtype), shb)
              for z in bzero]

    tk = _timed_min(fn, args, n)
    tb = _timed_min(fnb, argsb, n)
    print(f"  [wall: kernel {tk*1e3:.2f} ms, baseline {tb*1e3:.2f} ms]")
    return max(tk - tb, 0.0) * 1e9

